# revision 15
# baseline (speedup 1.0000x reference)
"""Trainium2 Bass kernel for nn_CrossAttention (B=8, N=4096, S=512, D=512, H=8).

Sharding: data-parallel over batch - each of the 8 NeuronCores computes the
full cross-attention for one batch element. No collectives needed.

v2 design (vs v1 baseline at ~420us):
  - bf16 matmuls for q/out projections (x, weights bf16 on host).
  - fp8e4(e4m3) + MatmulPerfMode.DoubleRow for the two K-starved attention
    matmuls (scores: K=64 packed as 32 partitions x 2; attnV: K=256 packed as
    128 partitions x 2 ctx chunks), halving PE streaming time there.
  - Host-side context compaction: context_mask is host-visible, so masked
    context positions (~50%) are dropped up front; S_c = padded-to-128 max
    effective length over the batch. Cuts kv-proj/scores/exp/attnV work.
  - Softmax denominator comes for free from a ones column in the attnV
    stationary (row 64 of the attnV PSUM). Denominators for all 8 heads are
    gathered into one [8, NT] tile, inverted with ONE [8, NT] reciprocal
    per query tile (v1 did full [128,512] nc.vector.reciprocal per head =
    107us; reciprocal_approx_fast hits an "ISA wrong length" codegen bug),
    then broadcast back over partitions via a DRAM bounce.
  - PSUM evacuations on DVE (gpsimd cannot access PSUM per the BIR
    verifier); normalization muls on the idle gpsimd (Pool) engine; exp
    stays on ACT; PE does only matmuls.
  - The output projection of tile t is emitted during tile t+1 (software
    pipeline) so the denominator round-trip latency never stalls the PE.
  - y is DMA'd straight from PSUM to DRAM.

Env overrides: KMMDT (bf16 default), KQK8=0 (disable fp8 scores), KEV8=0
(disable fp8 attnV).
"""

import os

import numpy as np

try:
    import concourse.bass as bass
except ImportError:
    import sys

    sys.path.insert(0, "/opt/trn_rl_repo")
    import concourse.bass as bass

from contextlib import ExitStack

import concourse.mybir as mybir
import concourse.tile as tile
from concourse.bass import ts

B, N, S, D, H = 8, 4096, 512, 512, 8
HD = D // H  # 64
SCALE = HD**-0.5
P = 128
IC = D // P  # 4 chunks of feature dims
NT = 512  # queries per outer tile
NTILES = N // NT  # 8
NSUB = NT // P  # 4
MASK_NEG = -30000.0

f32 = mybir.dt.float32
fp8 = mybir.dt.float8e4
DR = mybir.MatmulPerfMode.DoubleRow

MMDT_NAME = os.environ.get("KMMDT", "bfloat16")
# fp8 DoubleRow matmuls measured 0 per-instruction speedup on this HW (512
# cols stream at 1 col/cycle regardless), so fp8 scores only added error and
# 16 ACT-queue DMA issues/tile that stalled the PE -> default off
QK8 = os.environ.get("KQK8", "0") == "1"
# fp8 attnV alone costs ~4.5e-2 rel err (e+v quantization) vs the 2e-2
# tolerance -> keep attnV in bf16 by default
EV8 = os.environ.get("KEV8", "0") == "1"


def _np_mm(mmdt):
    return np.dtype(mybir.dt.np(mmdt))


def _split_multi_waits(nc: bass.Bass) -> None:
    """This walrus toolchain accepts at most ONE sync-wait per instruction
    ("Too many sync wait commands" in setupSyncWait, seen for MM/LW, NoOp,
    and DMA structs alike). Hoist all but the last wait of any instruction
    onto a chain of same-engine InstNoOps spliced immediately before it -
    same program position, so synchronization semantics are unchanged."""
    eng_map = {
        mybir.EngineType.PE: lambda: nc.tensor,
        mybir.EngineType.Activation: lambda: nc.scalar,
        mybir.EngineType.DVE: lambda: nc.vector,
        mybir.EngineType.Pool: lambda: nc.gpsimd,
        mybir.EngineType.SP: lambda: nc.sync,
    }
    for fn in nc.m.functions:
        blocks = fn.blocks
        for bb in blocks:
            insts = list(bb.instructions)
            out = []
            changed = False
            for inst in insts:
                si = inst.sync_info
                if (
                    si is not None
                    and len(si.on_wait) > 1
                    and inst.engine in eng_map
                ):
                    waits = list(si.on_wait)
                    for w in waits[:-1]:  # one nop per excess wait
                        nop = eng_map[inst.engine]().nop(nofuse=True).ins
                        # the nop was appended to whatever block is current;
                        # strip it from there before splicing it in place
                        for bb2 in blocks:
                            lst = list(bb2.instructions)
                            if any(x.name == nop.name for x in lst):
                                bb2.instructions = [
                                    x for x in lst if x.name != nop.name
                                ]
                                if bb2 is bb:
                                    insts = [
                                        x for x in insts if x.name != nop.name
                                    ]
                        nop.sync_info = mybir.SyncInfo(
                            on_wait=[w], on_update=[]
                        )
                        out.append(nop)
                    inst.sync_info = mybir.SyncInfo(
                        on_wait=waits[-1:], on_update=list(si.on_update)
                    )
                    changed = True
                out.append(inst)
            if changed:
                bb.instructions = out


def _build_nc(
    mmdt_name: str, qk8: bool, ev8: bool, SCc: int, has_bq, has_bk, has_bv, has_bp
) -> bass.Bass:
    mmdt = getattr(mybir.dt, mmdt_name)
    qkdt = fp8 if qk8 else mmdt
    evdt = fp8 if ev8 else mmdt
    Sc = SCc * P
    SCP = (SCc + 1) // 2  # attnV ctx pair-slots (last may be unpaired)

    nc = bass.Bass()

    xT = nc.dram_tensor("xT", [D, N], mmdt, kind="ExternalInput")
    ctxT = nc.dram_tensor("ctxT", [D, Sc], mmdt, kind="ExternalInput")
    wqT = nc.dram_tensor("wqT", [D, D], mmdt, kind="ExternalInput")
    wkT = nc.dram_tensor("wkT", [D, D], mmdt, kind="ExternalInput")
    wvT = nc.dram_tensor("wvT", [D, D], mmdt, kind="ExternalInput")
    wpT = nc.dram_tensor("wpT", [D, D], mmdt, kind="ExternalInput")
    bq = nc.dram_tensor("bq", [D, 1], f32, kind="ExternalInput")
    bk = nc.dram_tensor("bk", [D, 1], f32, kind="ExternalInput")
    bv = nc.dram_tensor("bv", [1, D], mmdt, kind="ExternalInput")
    bp = nc.dram_tensor("bp", [1, D], mmdt, kind="ExternalInput")
    amask = nc.dram_tensor("amask", [Sc, 1], f32, kind="ExternalInput")
    y = nc.dram_tensor("y", [N, D], f32, kind="ExternalOutput")

    rden_dram = nc.dram_tensor("rden_scratch", [NTILES, H, NT], mmdt)

    ch = lambda dram: dram.rearrange("(c p) o -> p c o", p=P)  # [P, IC, D]

    with tile.TileContext(nc) as tc, ExitStack() as ctx:
        const = ctx.enter_context(tc.tile_pool(name="const", bufs=1))
        work = ctx.enter_context(tc.tile_pool(name="work", bufs=2))
        epool = ctx.enter_context(tc.tile_pool(name="epool", bufs=8))
        psum = ctx.enter_context(tc.tile_pool(name="psum", bufs=1, space="PSUM"))

        # ---- persistent tiles -------------------------------------------
        wq_t = const.tile([P, IC, D], mmdt)
        wk_t = const.tile([P, IC, D], mmdt)
        wv_t = const.tile([P, IC, D], mmdt)
        wp_t = const.tile([P, IC, D], mmdt)
        ctx_t = const.tile([P, IC, Sc], mmdt)
        amask_t = const.tile([P, SCc, 1], f32)
        nc.sync.dma_start(wq_t[:], ch(wqT))
        nc.sync.dma_start(wk_t[:], ch(wkT))
        nc.sync.dma_start(wv_t[:], ch(wvT))
        nc.sync.dma_start(wp_t[:], ch(wpT))
        nc.sync.dma_start(ctx_t[:], ctxT.rearrange("(c p) s -> p c s", p=P))
        nc.sync.dma_start(amask_t[:], amask.rearrange("(c p) o -> p c o", p=P))

        if has_bq:
            bq_t = const.tile([P, IC, 1], f32)
            nc.sync.dma_start(bq_t[:], bq.rearrange("(c p) o -> p c o", p=P))
        if has_bk:
            bk_t = const.tile([P, IC, 1], f32)
            nc.sync.dma_start(bk_t[:], bk.rearrange("(c p) o -> p c o", p=P))
        if has_bv or has_bp:
            ones1_t = const.tile([1, P], mmdt)
            nc.vector.memset(ones1_t[:], 1.0)
        if has_bv:
            bv_t = const.tile([1, D], mmdt)
            nc.sync.dma_start(bv_t[:], bv[:])
        if has_bp:
            bp_t = const.tile([1, D], mmdt)
            nc.sync.dma_start(bp_t[:], bp[:])

        # keys, feature-major, in the scores matmul dtype
        if qk8:
            kf8_t = const.tile([P, IC, Sc], qkdt)  # flat fp8 keys
            # DoubleRow layout: head h lives at partitions 64*(h%2)..+32
            # (matmul operand base partition must be 0/32/64), j = h//2
            # selects the free block; i = K-half (hd 32i..32i+32)
            k8_t = const.tile([P, 4, SCc, 2, P], qkdt)
        else:
            kT_t = const.tile([P, IC, Sc], qkdt)
        # token-major v (+ ones col 64 for the softmax denominator), per head,
        # ctx chunks paired along free dim for DoubleRow (K = 2x128)
        vext_t = const.tile([P, H, SCP, 2, HD + 1], evdt)

        # ---- kv projections (once per core) -----------------------------
        for h in range(H):
            for scp in range(SCP):
                for i in range(2):
                    nc.vector.memset(vext_t[:, h, scp, i, HD : HD + 1], 1.0)

        for kc in range(IC):  # dk chunks -> kT (feature-major keys)
            ps = psum.tile([P, NT], f32, tag="ps_q", bufs=2)
            for i in range(IC):
                nc.tensor.matmul(
                    ps[:, 0:Sc],
                    wk_t[:, i, ts(kc, P)],
                    ctx_t[:, i, :],
                    start=(i == 0),
                    stop=(i == IC - 1),
                )
            kdst = kf8_t if qk8 else kT_t
            if has_bk:
                nc.vector.tensor_scalar_add(
                    kdst[:, kc, :], ps[:, 0:Sc], bk_t[:, kc, :]
                )
            else:
                nc.vector.tensor_copy(kdst[:, kc, :], ps[:, 0:Sc])
        if qk8:
            for kc in range(IC):
                for par in range(2):
                    h = 2 * kc + par
                    pb, j = 64 * (h % 2), h // 2
                    for i in range(2):
                        nc.sync.dma_start(
                            k8_t[pb : pb + 32, j, :, i, :],
                            kf8_t[
                                64 * par + 32 * i : 64 * par + 32 * i + 32, kc, :
                            ].rearrange("p (c s) -> p c s", s=P),
                        )

        for sc in range(SCc):  # s chunks -> v (token-major)
            ps = psum.tile([P, D], f32, tag="ps_y", bufs=1)
            for i in range(IC):
                nc.tensor.matmul(
                    ps[:],
                    ctx_t[:, i, ts(sc, P)],
                    wv_t[:, i, :],
                    start=(i == 0),
                    stop=(i == IC - 1 and not has_bv),
                )
            if has_bv:
                nc.tensor.matmul(ps[:], ones1_t[:], bv_t[:], start=False, stop=True)
            for h in range(H):
                nc.vector.tensor_copy(
                    vext_t[:, h, sc // 2, sc % 2, 0:HD],
                    ps[:, h * HD : (h + 1) * HD],
                )

        # ---- main loop over query tiles (outproj software-pipelined) ----
        prev = None  # (ot_t,) of previous tile
        for t in range(NTILES):
            xT_t = work.tile([P, IC, NT], mmdt, tag="xT")
            nc.sync.dma_start(
                xT_t[:], xT[:, ts(t, NT)].rearrange("(c p) n -> p c n", p=P)
            )

            # qT for this tile (feature-major, scores dtype)
            qm_t = work.tile([P, IC, NT], qkdt, tag="qm")
            for oc in range(IC):
                ps = psum.tile([P, NT], f32, tag="ps_q", bufs=2)
                for i in range(IC):
                    nc.tensor.matmul(
                        ps[:],
                        wq_t[:, i, ts(oc, P)],
                        xT_t[:, i, :],
                        start=(i == 0),
                        stop=(i == IC - 1),
                    )
                if has_bq:
                    nc.vector.tensor_scalar_add(qm_t[:, oc, :], ps[:], bq_t[:, oc, :])
                else:
                    nc.vector.tensor_copy(qm_t[:, oc, :], ps[:])
            if qk8:
                q8_t = work.tile([P, 4, 2, NT], qkdt, tag="q8")
                for oc in range(IC):
                    for par in range(2):
                        h = 2 * oc + par
                        pb, j = 64 * (h % 2), h // 2
                        for i in range(2):
                            nc.scalar.dma_start(
                                q8_t[pb : pb + 32, j, i, :],
                                qm_t[
                                    64 * par + 32 * i : 64 * par + 32 * i + 32,
                                    oc,
                                    :,
                                ],
                            )

            # scores + softmax-exp + attnV + denominator gather.
            # The den chain (recip -> DRAM bounce -> partition broadcast ->
            # muls) is split into two halves so each half's latency hides
            # under the PE work that follows it; outproj(t-1) is emitted
            # mid-iteration for the same reason.
            stag_t = work.tile([P, IC, NT], f32, tag="stag")
            dden_a = work.tile([2 * 2, NT], f32, tag="ddenA")
            dden_b = work.tile([2 * 2, NT], f32, tag="ddenB")
            den_t = work.tile([P, IC, NT], mmdt, tag="den")
            ot_t = work.tile([P, IC, NT], mmdt, tag="ot")

            def den_chain(half):
                dden_h = dden_a if half == 0 else dden_b
                eng = nc.sync if half == 0 else nc.scalar
                rd_t = work.tile([2 * 2, NT], f32, tag=f"rd{half}")
                rd16_t = work.tile([2 * 2, NT], mmdt, tag=f"rd16{half}")
                nc.vector.reciprocal(rd_t[:], dden_h[:])
                nc.vector.tensor_copy(rd16_t[:], rd_t[:])
                eng.dma_start(rden_dram[t, 4 * half : 4 * half + 4], rd16_t[:])
                for c in (2 * half, 2 * half + 1):
                    for par in (0, 1):
                        eng.dma_start(
                            den_t[par * HD : (par + 1) * HD, c, :],
                            rden_dram[
                                t, 2 * c + par : 2 * c + par + 1
                            ].to_broadcast((HD, NT)),
                        )
                    nc.gpsimd.tensor_mul(
                        ot_t[:, c, :], stag_t[:, c, :], den_t[:, c, :]
                    )

            for c in range(IC):  # head pairs (2c, 2c+1)
                for par in (0, 1):
                    h = 2 * c + par
                    e8 = epool.tile([P, SCP, 2, NT], evdt, tag="e")
                    for sc in range(SCc):
                        ps_s = psum.tile([P, NT], f32, tag="ps_s", bufs=3)
                        if qk8:
                            pb, j = 64 * (h % 2), h // 2
                            pg = slice(pb, pb + 32)
                            nc.tensor.matmul(
                                ps_s[:],
                                k8_t[pg, j, sc, :, :],
                                q8_t[pg, j, :, :],
                                start=True,
                                stop=True,
                                perf_mode=DR,
                            )
                        else:
                            pslc = slice(par * HD, (par + 1) * HD)
                            nc.tensor.matmul(
                                ps_s[:],
                                kT_t[pslc, c, ts(sc, P)],
                                qm_t[pslc, c, :],
                                start=True,
                                stop=True,
                            )
                        nc.scalar.activation(
                            e8[:, sc // 2, sc % 2, :],
                            ps_s[:],
                            mybir.ActivationFunctionType.Exp,
                            bias=amask_t[:, sc, :],
                            scale=SCALE,
                        )
                    # attnV: rows 0:64 = unnormalized out, row 64 = denom
                    ps_o = psum.tile(
                        [P, NT], f32, tag="ps_oe" if par == 0 else "ps_oo", bufs=1
                    )
                    for scp in range(SCP):
                        first = scp == 0
                        last = scp == SCP - 1
                        if 2 * scp + 1 < SCc and ev8:
                            nc.tensor.matmul(
                                ps_o[0 : HD + 1, :],
                                vext_t[:, h, scp, :, :],
                                e8[:, scp, :, :],
                                start=first,
                                stop=last,
                                perf_mode=DR,
                            )
                        elif 2 * scp + 1 < SCc:
                            for i in (0, 1):
                                nc.tensor.matmul(
                                    ps_o[0 : HD + 1, :],
                                    vext_t[:, h, scp, i, :],
                                    e8[:, scp, i, :],
                                    start=first and i == 0,
                                    stop=last and i == 1,
                                )
                        else:  # unpaired tail chunk
                            nc.tensor.matmul(
                                ps_o[0 : HD + 1, :],
                                vext_t[:, h, scp, 0, :],
                                e8[:, scp, 0, :],
                                start=first,
                                stop=True,
                            )
                    # evacuate PSUM (DMA can't read PSUM): gpsimd copy to
                    # SBUF, then SBUF->SBUF DMAs shift the odd head to
                    # partitions 64:128 and gather the denominator rows
                    oe = epool.tile([HD + 1, NT], f32, tag="oe")
                    if par == 0:
                        nc.vector.tensor_copy(stag_t[0:HD, c, :], ps_o[0:HD, :])
                        nc.vector.tensor_copy(
                            oe[HD : HD + 1, :], ps_o[HD : HD + 1, :]
                        )
                    else:
                        nc.vector.tensor_copy(oe[:], ps_o[0 : HD + 1, :])
                        nc.sync.dma_start(stag_t[HD:P, c, :], oe[0:HD, :])
                    hh = h - 4 * (c // 2)
                    dden_h = dden_a if c < 2 else dden_b
                    deng = nc.sync if c < 2 else nc.scalar
                    deng.dma_start(
                        dden_h[hh : hh + 1, :], oe[HD : HD + 1, :]
                    )
                # after pair 1: first den half-chain; then outproj(t-1)
                if c == 1:
                    den_chain(0)
                    if prev is not None:
                        _emit_outproj(nc, psum, epool, prev[0], wp_t, y,
                                      prev[1], has_bp,
                                      ones1_t if has_bp else None,
                                      bp_t if has_bp else None)
                elif c == 3:
                    den_chain(1)

            prev = (ot_t, t)

        _emit_outproj(nc, psum, epool, prev[0], wp_t, y, prev[1], has_bp,
                      ones1_t if has_bp else None, bp_t if has_bp else None)

    _split_multi_waits(nc)
    return nc


def _emit_outproj(nc, psum, epool, ot_t, wp_t, y, t, has_bp, ones1_t, bp_t):
    for ns in range(NSUB):
        ps_y = psum.tile([P, D], f32, tag="ps_y", bufs=1)
        for c in range(IC):
            nc.tensor.matmul(
                ps_y[:],
                ot_t[:, c, ts(ns, P)],
                wp_t[:, c, :],
                start=(c == 0),
                stop=(c == IC - 1 and not has_bp),
            )
        if has_bp:
            nc.tensor.matmul(ps_y[:], ones1_t[:], bp_t[:], start=False, stop=True)
        y_t = epool.tile([P, D], f32, tag="y")
        nc.vector.tensor_copy(y_t[:], ps_y[:])
        nc.sync.dma_start(
            y[t * NT + ns * P : t * NT + (ns + 1) * P, :], y_t[:]
        )


_NC_CACHE: dict = {}


def _get_nc(flags):
    if flags not in _NC_CACHE:
        _NC_CACHE[flags] = _build_nc(*flags)
    return _NC_CACHE[flags]


def _prep_in_maps(x, context, context_mask, wq, bq, wkv, bkv, wp, bp,
                  mmdt_name=None):
    if mmdt_name is None:
        mmdt_name = MMDT_NAME
    np_mm = _np_mm(getattr(mybir.dt, mmdt_name))
    cvt = lambda a: np.ascontiguousarray(a).astype(np_mm, copy=False)

    # context compaction: unmasked positions first, truncate to the padded
    # max effective length over the batch (mask True = padding)
    n_eff = (~context_mask).sum(axis=1)
    Sc = int(min(S, max(P, -(-int(n_eff.max()) // P) * P)))
    SCc = Sc // P

    wqT = cvt(wq.T)
    wkT = cvt(wkv[:D].T)
    wvT = cvt(wkv[D:].T)
    wpT = cvt(wp.T)
    bq_c = np.ascontiguousarray(bq.reshape(D, 1), dtype=np.float32)
    bk_c = np.ascontiguousarray(bkv[:D].reshape(D, 1), dtype=np.float32)
    bv_r = cvt(bkv[D:].reshape(1, D))
    bp_r = cvt(bp.reshape(1, D))
    flags = (
        mmdt_name,
        QK8,
        EV8,
        SCc,
        bool(np.any(bq != 0)),
        bool(np.any(bkv[:D] != 0)),
        bool(np.any(bkv[D:] != 0)),
        bool(np.any(bp != 0)),
    )
    in_maps = []
    for b in range(B):
        sel = np.argsort(context_mask[b], kind="stable")[:Sc]
        ctx_c = context[b][sel]
        amask_c = np.where(
            context_mask[b][sel], np.float32(MASK_NEG), np.float32(0.0)
        )
        in_maps.append(
            {
                "xT": cvt(x[b].T),
                "ctxT": cvt(ctx_c.T),
                "wqT": wqT,
                "wkT": wkT,
                "wvT": wvT,
                "wpT": wpT,
                "bq": bq_c,
                "bk": bk_c,
                "bv": bv_r,
                "bp": bp_r,
                "amask": amask_c.astype(np.float32).reshape(Sc, 1),
            }
        )
    return in_maps, flags


def kernel(x, context, context_mask, wq, bq, wkv, bkv, wp, bp):
    from concourse.bass_utils import run_bass_kernel_spmd

    in_maps, flags = _prep_in_maps(
        x, context, context_mask, wq, bq, wkv, bkv, wp, bp
    )
    nc = _get_nc(flags)
    res = run_bass_kernel_spmd(nc, in_maps, list(range(B)))
    return np.stack([np.asarray(res.results[b]["y"]) for b in range(B)], axis=0)
